# revision 31
# baseline (speedup 1.0000x reference)
"""CodeGen-style attention block, tensor-parallel over heads on 8 Trainium2 cores.

v2: all-bf16 storage/matmuls (f32 PSUM accumulate), serial phase structure
P0 -> A0 -> P1 -> A1 -> OP:
  - P(b): single-pass QKV projection for batch b, 256-token windows, all 12
    weight column-tiles resident (bf16 halves SBUF; FWL hides LDWEIGHTS).
    k and v are written straight into persistent SBUF tiles (no DRAM
    round-trip); q goes to DRAM and is read back per query-block.  v is
    computed directly in [tok, d] form by swapping the matmul operands
    (hsT chunk stationary, v-weights moving) - no PE transposes.
  - A(b): causal attention for this core's 2 heads.  Scores/exp/AV/den with
    diagonal narrowing (only the live column range of diagonal k-chunks is
    computed) and depth-2 software pipelining of score vs AV matmuls in the
    PE stream.  Normalization is deferred: unnormalized AV sums (bf16) and
    softmax denominators (f32) are written to DRAM.
  - OP: out-projection.  Loads AV/den back, reciprocal (fast approx) +
    normalize on DVE off the critical path, then the w_out row-slice matmuls.
Host sums the 8 partial [tokens, H] outputs (out-proj head contraction).
"""

import sys
import types
from contextlib import ExitStack

import numpy as np
import ml_dtypes

import concourse.bacc as bacc
import concourse.mybir as mybir
import concourse.tile as tile
from concourse.bass_utils import run_bass_kernel_spmd

try:
    import antenv.axon_hooks  # noqa: F401
except ImportError:
    _stub = types.ModuleType("antenv.axon_hooks")
    _stub.get_axon_ntff_profile_hook = lambda: None
    _stub.set_axon_ntff_profile_hook = lambda h: None
    sys.modules.setdefault("antenv.axon_hooks", _stub)

F32 = mybir.dt.float32
BF = mybir.dt.bfloat16
AF = mybir.ActivationFunctionType
NPBF = ml_dtypes.bfloat16

B, S, H = 2, 2048, 4096
N_HEAD, HEAD_DIM, ROT = 16, 256, 64
MAX_POS = 2048
TOK = B * S            # 4096
N_CORES = 8
HPC = N_HEAD // N_CORES  # heads per core = 2
DPC = HPC * HEAD_DIM     # dims per core = 512
NEG = -30000.0
NW = 16                  # 256-token windows

LAST_EXEC_NS = None
_NC_CACHE = []


def _build():
    nc = bacc.Bacc("TRN2", target_bir_lowering=False, debug=False,
                   num_devices=N_CORES)

    # [w, p(H-chunk), hc*256 + t]: hidden^T window tiles
    hst_d = nc.dram_tensor("hst", [NW, 128, 32 * 256], BF, kind="ExternalInput")
    # [oc, p(H-chunk), hc*128 + d]: q ocs 0..3 then k ocs 4..7 (stationary)
    wqkv_d = nc.dram_tensor("wqkv", [8, 128, 32 * 128], BF, kind="ExternalInput")
    # [p(H-chunk), hc*512 + vd]: v weights as the moving operand
    wv_d = nc.dram_tensor("wv", [128, 32 * 512], BF, kind="ExternalInput")
    rope_d = nc.dram_tensor("rope", [128, TOK], F32, kind="ExternalInput")
    rt_d = nc.dram_tensor("rt", [64, 64], BF, kind="ExternalInput")
    onm_d = nc.dram_tensor("onesm", [128, 128], BF, kind="ExternalInput")
    tri_d = nc.dram_tensor("tri", [128, 128], F32, kind="ExternalInput")
    kb_d = nc.dram_tensor("kb", [128, 32], F32, kind="ExternalInput")
    # [p, ci(=2*hl+dc), h]: per-core w_out row-slice
    wout_d = nc.dram_tensor("wout", [128, 4, H], BF, kind="ExternalInput")
    out_d = nc.dram_tensor("out", [TOK, H], BF, kind="ExternalOutput")

    with tile.TileContext(nc) as tc:
        with ExitStack() as st0:
            ec0 = st0.enter_context
            dram_pool = ec0(tc.tile_pool(name="dram", bufs=1, space="DRAM"))
            # q_d[(b, qt)]: [128, oc, 512] -- per-qt tiles so an attention
            # block's load only RAW-depends on its own two windows' stores
            q_d = {}
            for b in range(B):
                for qt in range(4):
                    q_d[(b, qt)] = dram_pool.tile(
                        [128, 4, 512], BF, tag=f"qd{b}{qt}", name=f"qd{b}{qt}")
            # b=0 only: b=1 keeps avc/den resident in SBUF through OP
            avc_d = {}
            den_d = {}
            ao_d = {}
            for qt in range(4):
                den_d[(0, qt)] = dram_pool.tile(
                    [128, 2, 512], F32, tag=f"dnd0{qt}", name=f"dnd0{qt}")
                avc_d[(0, qt)] = dram_pool.tile(
                    [128, 4, 512], BF, tag=f"avd0{qt}", name=f"avd0{qt}")
                # b=0 attn output, normalized during P1 (DVE/gpsimd idle
                # there) so A1/OP never spend reciprocal work on b=0
                ao_d[(0, qt)] = dram_pool.tile(
                    [128, 4, 512], BF, tag=f"aod0{qt}", name=f"aod0{qt}")

            # constants (small) -- on the scalar HWDGE queue so the sync
            # queue starts on the weight tiles immediately
            c0 = ec0(tc.tile_pool(name="consts", bufs=1))
            rt_sb = c0.tile([64, 64], BF)
            nc.scalar.dma_start(rt_sb[:], rt_d[:])
            kb_sb = c0.tile([128, 32], F32)
            nc.scalar.dma_start(kb_sb[:], kb_d[:])
            tri_sb = c0.tile([128, 128], F32)
            nc.scalar.dma_start(tri_sb[:], tri_d[:])
            onm_sb = c0.tile([128, 128], BF)
            nc.scalar.dma_start(onm_sb[:], onm_d[:])
            # preload the exp ACT table under phase-0 compute (one tiny exp)
            dummy_sb = c0.tile([1, 8], BF)
            nc.scalar.activation(dummy_sb[:], kb_sb[0:1, 0:8], AF.Exp)

            # persistent per-batch attention inputs (reused b0 -> b1)
            kvp = ec0(tc.tile_pool(name="kv", bufs=1))
            kts = {(hl, dc): kvp.tile([128, 2048], BF, tag=f"kt{hl}{dc}",
                                      name=f"kt{hl}{dc}")
                   for hl in range(2) for dc in range(2)}
            vh = kvp.tile([128, 16 * 512], BF, tag="vh", name="vh")

            # ---------------- attention machinery ----------------
            qpool = ec0(tc.tile_pool(name="qq", bufs=2))

            def q_dmas(b, qt, hl):
                q = qpool.tile([128, 2, 512], BF, tag="q")
                nc.sync.dma_start(q[:], q_d[(b, qt)][:, 2 * hl:2 * hl + 2, :])
                return q

            def attn_phase(ast, b, preq=None, hook=None, sink=None):
                ec = ast.enter_context
                # 8 PSUM banks: sc 3 + av 2x2 + dn 1.  av double-buffered so
                # a new block's AV matmuls never wait on the previous block's
                # DVE drain; den is written+drained in one deferred pipe slot
                # so a single bank suffices.
                scpool = ec(tc.tile_pool(name=f"sc{b}", bufs=3, space="PSUM"))
                avpool = ec(tc.tile_pool(name=f"av{b}", bufs=2, space="PSUM"))
                dnpool = ec(tc.tile_pool(name=f"dn{b}", bufs=1, space="PSUM"))
                expool = ec(tc.tile_pool(name=f"ex{b}", bufs=6))
                aspool = ec(tc.tile_pool(name=f"as{b}", bufs=1))
                xspool = ec(tc.tile_pool(name=f"xs{b}", bufs=2))

                blocks = [(qt, hl) for qt in range(4) for hl in range(2)]
                qtiles = {}

                def load_q(blk):
                    qtiles[blk] = q_dmas(b, blk[0], blk[1])

                if preq is not None:
                    qtiles[blocks[0]] = preq
                else:
                    load_q(blocks[0])

                # cross-block pipelined stream: sc/tri/exp run LAG chunks
                # ahead of the av/den matmuls, flowing across block
                # boundaries so the PE never drains at a block switch.
                LAG = 4
                pipe = []  # (emit_avden, fin_or_None)

                def pump():
                    emit, fin = pipe.pop(0)
                    emit()
                    if fin is not None:
                        fin()

                for bi, blk in enumerate(blocks):
                    qt, hl = blk
                    if hook is not None:
                        hook(bi)
                    if bi + 1 < len(blocks):
                        load_q(blocks[bi + 1])
                    qs = qtiles.pop(blk)
                    nkc = 4 * qt + 4
                    av0 = avpool.tile([128, 512], F32, tag="av0")
                    av1 = avpool.tile([128, 512], F32, tag="av1")
                    # ALL exp chunks (diag included, over their live col
                    # range) are elementwise-accumulated on the DVE; a single
                    # full-width den matmul per block folds the partition sum.
                    exsum = xspool.tile([128, 512], F32, tag="xs")
                    exsum_bf = xspool.tile([128, 512], BF, tag="xsb")

                    def make_fin_av(qt=qt, hl=hl, av0=av0, av1=av1):
                        def fin():
                            if sink is not None:
                                avsb, _ = sink
                                for dc, av in ((0, av0), (1, av1)):
                                    nc.vector.tensor_copy(
                                        avsb[qt][:, 2 * hl + dc, :], av[:])
                            else:
                                avs = aspool.tile([128, 2, 512], BF,
                                                  tag="avs")
                                for dc, av in ((0, av0), (1, av1)):
                                    nc.vector.tensor_copy(
                                        avs[:, dc, :], av[:])
                                nc.gpsimd.dma_start(
                                    avc_d[(b, qt)][:, 2 * hl:2 * hl + 2, :],
                                    avs[:])
                        return fin

                    def make_den(qt=qt, hl=hl, exsum_bf=exsum_bf):
                        # deferred a pipe slot past the block's last chunk so
                        # the DVE exsum chain never gates the PE
                        def emit():
                            den = dnpool.tile([128, 512], F32, tag="den")
                            nc.tensor.matmul(den[:], onm_sb[:], exsum_bf[:],
                                             start=True, stop=True)
                            if sink is not None:
                                _, densb = sink
                                nc.vector.tensor_copy(
                                    densb[qt][:, hl, :], den[:])
                            else:
                                dns = aspool.tile([128, 512], F32, tag="dns")
                                nc.vector.tensor_copy(dns[:], den[:])
                                nc.gpsimd.dma_start(
                                    den_d[(b, qt)][:, hl, :], dns[:])
                        return emit

                    for kc in range(nkc):
                        j = kc - 4 * qt
                        n0 = 128 * j if j > 0 else 0
                        sc = scpool.tile([128, 512], F32, tag="sc")
                        nc.tensor.matmul(
                            sc[:, n0:512], kts[(hl, 0)][:, kc * 128:(kc + 1) * 128],
                            qs[:, 0, n0:512], start=True, stop=False)
                        nc.tensor.matmul(
                            sc[:, n0:512], kts[(hl, 1)][:, kc * 128:(kc + 1) * 128],
                            qs[:, 1, n0:512], start=False, stop=True)
                        if j >= 0:
                            # causal triangle within the diagonal 128-col band
                            nc.vector.tensor_add(
                                sc[:, n0:n0 + 128], sc[:, n0:n0 + 128], tri_sb[:])
                        ex = expool.tile([128, 512], BF, tag="ex")
                        nc.scalar.activation(
                            ex[:, n0:512], sc[:, n0:512], AF.Exp,
                            scale=1.0 / 16.0,
                            bias=kb_sb[:, b * 16 + kc:b * 16 + kc + 1])
                        if kc == 0:
                            nc.vector.tensor_copy(exsum[:], ex[:])
                        else:
                            nc.vector.tensor_add(
                                exsum[:, n0:512], exsum[:, n0:512],
                                ex[:, n0:512])
                        if kc == nkc - 1:
                            nc.vector.tensor_copy(exsum_bf[:], exsum[:])

                        def emit(ex=ex, n0=n0, kc=kc, nkc=nkc, hl=hl,
                                 av0=av0, av1=av1):
                            st, sp = (kc == 0), (kc == nkc - 1)
                            base = kc * 512 + hl * 256
                            nc.tensor.matmul(
                                av0[:, n0:512], vh[:, base:base + 128],
                                ex[:, n0:512], start=st, stop=sp)
                            nc.tensor.matmul(
                                av1[:, n0:512], vh[:, base + 128:base + 256],
                                ex[:, n0:512], start=st, stop=sp)

                        pipe.append(
                            (emit, make_fin_av() if kc == nkc - 1 else None))
                        if len(pipe) > LAG:
                            pump()
                    pipe.append((make_den(), None))
                    if len(pipe) > LAG:
                        pump()
                while pipe:
                    pump()

            # ---------------- phases P0, A0, P1 ----------------
            with ExitStack() as stw:
                ecw = stw.enter_context
                wpool = ecw(tc.tile_pool(name="w", bufs=1))
                htpool = ecw(tc.tile_pool(name="ht", bufs=2))
                ropool = ecw(tc.tile_pool(name="ro", bufs=2))
                spool = ecw(tc.tile_pool(name="stage", bufs=4))
                tpool = ecw(tc.tile_pool(name="rott", bufs=2))

                def ht_load(w, strips=1):
                    t = htpool.tile([128, 32 * 256], BF, tag="ht", name="ht")
                    step = 32 // strips
                    for s in range(strips):
                        cs = slice(s * step * 256, (s + 1) * step * 256)
                        nc.sync.dma_start(t[:, cs], hst_d[w][:, cs])
                    return t

                def rope_load(w):
                    t = ropool.tile([128, 256], F32, tag="ro", name="ro")
                    nc.sync.dma_start(t[:], rope_d[:, w * 256:(w + 1) * 256])
                    return t

                # startup cascade: the first k-chain consumes wt4/ht in
                # hc order, so stripe both and interleave the DMAs to get
                # the first matmul issuing as early as possible.
                wts = {}
                wt4 = wpool.tile([128, 32 * 128], BF, tag="w4", name="wt4")
                wts[4] = wt4
                ht0 = htpool.tile([128, 32 * 256], BF, tag="ht", name="ht")
                # small leading strips so the first k-chain's leading hc
                # tiles land (and the chain starts) as early as possible
                nc.sync.dma_start(wt4[:, 0:1024], wqkv_d[4][:, 0:1024])
                nc.sync.dma_start(ht0[:, 0:2048], hst_d[0][:, 0:2048])
                nc.sync.dma_start(wt4[:, 1024:4096], wqkv_d[4][:, 1024:4096])
                nc.sync.dma_start(ht0[:, 2048:8192], hst_d[0][:, 2048:8192])
                for kq in range(1, 4):
                    wt = wpool.tile([128, 32 * 128], BF, tag=f"w{4 + kq}",
                                    name=f"wt{4 + kq}")
                    nc.sync.dma_start(wt[:], wqkv_d[4 + kq])
                    wts[4 + kq] = wt
                pre = (ht0, rope_load(0))
                # q weights BEFORE wv: window 0 runs k -> q -> v so the wv
                # transfer hides behind the q chains
                for oc in range(4):
                    wt = wpool.tile([128, 32 * 128], BF, tag=f"w{oc}",
                                    name=f"wt{oc}")
                    nc.sync.dma_start(wt[:], wqkv_d[oc])
                    wts[oc] = wt
                wv_sb = wpool.tile([128, 32 * 512], BF, tag="wv", name="wv")
                nc.sync.dma_start(wv_sb[:], wv_d[:])

                # warm the PE (HAM) under the startup DMA backlog
                with ExitStack() as wst:
                    wps = wst.enter_context(
                        tc.tile_pool(name="warm", bufs=1, space="PSUM"))
                    warm = wps.tile([64, 64], F32)
                    for i in range(64):
                        nc.tensor.matmul(warm[:], rt_sb[:], rt_sb[:],
                                         start=(i == 0), stop=(i == 63))

                # b=0 out-proj normalization, precomputed under P1:
                # ao = avc * (1/den), all off the PE / critical engines
                def norm_prep0(qt, nppool):
                    # loads ride the sync queue: their data lands a window
                    # later, so the scheduler cannot hoist the DVE recips
                    # into the A0 drain window
                    al = nppool.tile([128, 4, 512], BF, tag="nal")
                    nc.sync.dma_start(al[:], avc_d[(0, qt)][:])
                    dn = nppool.tile([128, 2, 512], F32, tag="ndn")
                    nc.sync.dma_start(dn[:], den_d[(0, qt)][:])
                    ao = nppool.tile([128, 4, 512], BF, tag="nao")
                    for hl in range(2):
                        rc = nppool.tile([128, 512], F32, tag="nrc")
                        scr = nppool.tile([128, 512], F32, tag="nscr")
                        nc.vector.reciprocal_approx_accurate(
                            out=rc[:], in_=dn[:, hl, :], scratch=scr[:])
                        for dc in range(2):
                            nc.gpsimd.tensor_mul(
                                ao[:, 2 * hl + dc, :],
                                al[:, 2 * hl + dc, :], rc[:])
                    nc.gpsimd.dma_start(ao_d[(0, qt)][:], ao[:])

                def proj_phase(pst, b, pre):
                    """QKV projection for batch b (windows 8b .. 8b+7)."""
                    ec = pst.enter_context
                    apool = ec(tc.tile_pool(name=f"pa{b}", bufs=3, space="PSUM"))
                    rpool = ec(tc.tile_pool(name=f"pr{b}", bufs=2, space="PSUM"))
                    nppool = (ec(tc.tile_pool(name="nprep", bufs=1))
                              if b == 1 else None)
                    ht, ro = pre
                    nxt = None
                    preq = None
                    for wl in range(8):
                        w = 8 * b + wl
                        if wl > 0:
                            ht, ro = next_pre
                        if wl == 2:
                            # early q loads for this batch's first attn block
                            preq = q_dmas(b, 0, 0)

                        def rotary(dst, acc, stage_rows, ro=ro):
                            # dst[0:64] <- acc*cos + rotate_every_two(acc)*sin
                            rp = rpool.tile([64, 256], F32)
                            nc.tensor.matmul(rp[:], rt_sb[:], stage_rows,
                                             start=True, stop=True)
                            t1 = tpool.tile([64, 256], F32, tag="t1")
                            nc.vector.tensor_mul(t1[:], acc[0:64, :], ro[0:64, :])
                            t2 = tpool.tile([64, 256], F32, tag="t2")
                            nc.vector.tensor_mul(t2[:], rp[:], ro[64:128, :])
                            nc.vector.tensor_add(dst, t1[:], t2[:])

                        # k ocs (stationary weights, transposed domain)
                        for kq in range(4):
                            hl, dc = kq // 2, kq % 2
                            acc = apool.tile([128, 256], F32, tag="acc")
                            for hc in range(32):
                                nc.tensor.matmul(
                                    acc[:],
                                    wts[4 + kq][:, hc * 128:(hc + 1) * 128],
                                    ht[:, hc * 256:(hc + 1) * 256],
                                    start=(hc == 0), stop=(hc == 31))
                            dst = kts[(hl, dc)][:, wl * 256:(wl + 1) * 256]
                            nc.vector.tensor_copy(dst, acc[:])
                            if dc == 0:
                                rotary(dst[0:64, :], acc, dst[0:64, :])
                            if kq == 1:
                                # prefetch next window under this one
                                if wl < 7:
                                    next_pre = (ht_load(w + 1), rope_load(w + 1))
                                elif b == 0:
                                    nxt = (ht_load(8), rope_load(8))
                        def v_chains():
                            # v (hsT chunks stationary, v-weights moving
                            # -> [tok, d])
                            for ts in range(2):
                                acc = apool.tile([128, 512], F32, tag="acc")
                                for hc in range(32):
                                    nc.tensor.matmul(
                                        acc[:],
                                        ht[:, hc * 256 + ts * 128:
                                           hc * 256 + (ts + 1) * 128],
                                        wv_sb[:, hc * 512:(hc + 1) * 512],
                                        start=(hc == 0), stop=(hc == 31))
                                kc = wl * 2 + ts
                                nc.vector.tensor_copy(
                                    vh[:, kc * 512:(kc + 1) * 512], acc[:])

                        def q_chains():
                            # q ocs -> one staged tile -> single DMA
                            stage = spool.tile([128, 4, 256], BF, tag="qs")
                            for oc in range(4):
                                hl, dc = oc // 2, oc % 2
                                acc = apool.tile([128, 256], F32, tag="acc")
                                for hc in range(32):
                                    nc.tensor.matmul(
                                        acc[:],
                                        wts[oc][:, hc * 128:(hc + 1) * 128],
                                        ht[:, hc * 256:(hc + 1) * 256],
                                        start=(hc == 0), stop=(hc == 31))
                                dst = stage[:, oc, :]
                                nc.vector.tensor_copy(dst, acc[:])
                                if dc == 0:
                                    rotary(dst[0:64, :], acc, dst[0:64, :])
                            nc.sync.dma_start(
                                q_d[(b, wl // 2)][:, :, (wl % 2) * 256:
                                                  (wl % 2) * 256 + 256],
                                stage[:])

                        if b == 0 and wl == 0:
                            q_chains()
                            v_chains()
                        else:
                            v_chains()
                            q_chains()
                        if b == 1 and 2 <= wl <= 5:
                            norm_prep0(wl - 2, nppool)
                    return nxt, preq

                with ExitStack() as pst:
                    pre1, preq0 = proj_phase(pst, 0, pre)
                with ExitStack() as ast:
                    attn_phase(ast, 0, preq=preq0)
                with ExitStack() as pst:
                    _, preq1 = proj_phase(pst, 1, pre1)
            # w_scope closed: projection SBUF freed

            with ExitStack() as st3:
                ec3 = st3.enter_context
                c3 = ec3(tc.tile_pool(name="wo", bufs=1))
                wout_sb = c3.tile([128, 4, H], BF)
                # b=1 avc/den never leave SBUF
                avsb = {qt: c3.tile([128, 4, 512], BF, tag=f"avsb{qt}",
                                    name=f"avsb{qt}") for qt in range(4)}
                densb = {qt: c3.tile([128, 2, 512], F32, tag=f"dnsb{qt}",
                                     name=f"dnsb{qt}") for qt in range(4)}

                # ---------------- out-projection ----------------
                alpool = ec3(tc.tile_pool(name="al", bufs=3))
                recpool = ec3(tc.tile_pool(name="rec", bufs=3))
                scrpool = ec3(tc.tile_pool(name="scr", bufs=2))
                aopool = ec3(tc.tile_pool(name="ao", bufs=4))
                ospool = ec3(tc.tile_pool(name="os", bufs=2))

                aos_all = {}

                def prep(b, qt, eng=None):
                    if b == 0:
                        # normalization already done under P1 -- pure load.
                        # sync queue: its completion signal reaches the PE
                        # directly instead of riding the busy ACT stream
                        aol = alpool.tile([128, 4, 512], BF, tag="al")
                        nc.sync.dma_start(aol[:], ao_d[(0, qt)][:])
                        aos_all[(b, qt)] = lambda hl, dc: aol[:, 2 * hl + dc, :]
                        return
                    al, dn = avsb[qt], densb[qt]
                    aos = {}
                    for hl in range(2):
                        rc = recpool.tile([128, 512], F32, tag=f"rc{hl}")
                        scr = scrpool.tile([128, 512], F32, tag="scr")
                        nc.vector.reciprocal_approx_accurate(
                            out=rc[:], in_=dn[:, hl, :], scratch=scr[:])
                        for dc in range(2):
                            ao = aopool.tile([128, 512], BF, tag=f"ao{hl}{dc}")
                            (eng or nc.vector).tensor_mul(
                                ao[:], al[:, 2 * hl + dc, :], rc[:])
                            aos[(hl, dc)] = ao
                    aos_all[(b, qt)] = lambda hl, dc: aos[(hl, dc)][:]

                def op_block(b, qt):
                    aos = aos_all.pop((b, qt))
                    for tc_ in range(4):
                        stage = ospool.tile([128, 8 * 512], BF, tag="os")
                        for ht_ in range(8):
                            op = oppool.tile([128, 512], F32, tag="op")
                            for ci, (hl, dc) in enumerate(
                                    ((0, 0), (0, 1), (1, 0), (1, 1))):
                                nc.tensor.matmul(
                                    op[:],
                                    aos(hl, dc)[:, tc_ * 128:(tc_ + 1) * 128],
                                    wout_sb[:, 2 * hl + dc,
                                            ht_ * 512:(ht_ + 1) * 512],
                                    start=(ci == 0), stop=(ci == 3))
                            dst = stage[:, ht_ * 512:(ht_ + 1) * 512]
                            if ht_ % 2 == 0:
                                nc.vector.tensor_copy(dst, op[:])
                            else:
                                nc.scalar.copy(dst, op[:])
                        r0 = b * 2048 + qt * 512 + tc_ * 128
                        nc.sync.dma_start(out_d[r0:r0 + 128, :], stage[:])

                def a1_hook(bi):
                    # all hook work is pure DMA prefetch on the sync queue
                    # (idle in A1 but for one q load per block), starting at
                    # bi=1 so the first dispatch's SBUF anti-dependency (on
                    # P1's last q-store) is already clear
                    if bi in (1, 2, 3, 4):
                        i = bi - 1
                        nc.sync.dma_start(wout_sb[:, i, :],
                                          wout_d[:, i, :])
                    elif bi in (5, 6, 7):
                        prep(0, bi - 5)

                with ExitStack() as ast:
                    attn_phase(ast, 1, preq=preq1, hook=a1_hook,
                               sink=(avsb, densb))

                oppool = ec3(tc.tile_pool(name="op", bufs=4, space="PSUM"))
                obs = [(b, qt) for b in range(B) for qt in range(4)]
                for i, ob in enumerate(obs):
                    op_block(*ob)
                    if i + 3 < len(obs):
                        prep(*obs[i + 3])
    nc.compile()
    return nc


def _get_nc():
    if not _NC_CACHE:
        _NC_CACHE.append(_build())
    return _NC_CACHE[0]


def _host_prep(hidden_states, position_ids, attention_mask, w_qkv, w_out):
    hid = np.ascontiguousarray(np.asarray(hidden_states, np.float32)).reshape(TOK, H)
    w_qkv = np.asarray(w_qkv, np.float32)
    w_out = np.asarray(w_out, np.float32)
    pos = np.asarray(position_ids).astype(np.int64)
    am = np.asarray(attention_mask).reshape(B, S).astype(bool)

    # hsT window tiles [w, p, hc*256 + t]
    hst = np.ascontiguousarray(
        hid.astype(NPBF).reshape(NW, 256, 32, 128).transpose(0, 3, 2, 1)
    ).reshape(NW, 128, 32 * 256)

    # rotary tables, matching reference.create_sinusoidal_positions
    inv_freq = 1.0 / 10000 ** (np.arange(0, ROT, 2) / ROT)
    si = np.einsum('i,j->ij', np.arange(MAX_POS), inv_freq).astype('float32')
    emb = np.concatenate([np.sin(si), np.cos(si)], axis=-1)  # [2048, 64]
    sincos = emb[pos]                    # [B, S, 64]
    sin_rep = np.repeat(sincos[..., :ROT // 2], 2, axis=2)   # [B, S, 64]
    cos_rep = np.repeat(sincos[..., ROT // 2:], 2, axis=2)
    rope = np.empty((128, TOK), np.float32)
    rope[0:64] = cos_rep.reshape(TOK, 64).T
    rope[64:128] = sin_rep.reshape(TOK, 64).T

    rt = np.zeros((64, 64), np.float32)
    rt[np.arange(1, 64, 2), np.arange(0, 64, 2)] = -1.0
    rt[np.arange(0, 64, 2), np.arange(1, 64, 2)] = 1.0

    onesm = np.ones((128, 128), np.float32)

    p_idx = np.arange(128)[:, None]
    c_idx = np.arange(128)[None, :]
    tri = np.where(p_idx <= c_idx, 0.0, NEG).astype(np.float32)

    kb = np.where(am.reshape(B, 16, 128), 0.0, NEG).astype(
        np.float32).transpose(2, 0, 1).reshape(128, 32)
    kb = np.ascontiguousarray(kb)

    shared = dict(hst=hst, rope=rope, rt=rt.astype(NPBF),
                  onesm=onesm.astype(NPBF), tri=tri, kb=kb)

    in_maps = []
    for c in range(N_CORES):
        # q ocs 0..3 then k ocs 4..7; fused layout per mp-group is (q, v, k)
        occols = []
        for part in (0, 2):  # 0 = query, 2 = key
            for hl in range(HPC):
                h = HPC * c + hl
                base = (h // 4) * 3072 + part * 1024 + (h % 4) * 256
                occols.append(np.arange(base, base + 256))
        occols = np.concatenate(occols)  # [1024] = q(512) | k(512)
        wslice = w_qkv[:, occols].astype(NPBF)  # [4096, 1024]
        wqkv_prep = np.ascontiguousarray(
            wslice.reshape(32, 128, 8, 128).transpose(2, 1, 0, 3)
        ).reshape(8, 128, 32 * 128)

        vcols = []
        for hl in range(HPC):
            h = HPC * c + hl
            base = (h // 4) * 3072 + 1 * 1024 + (h % 4) * 256
            vcols.append(np.arange(base, base + 256))
        vcols = np.concatenate(vcols)    # [512]
        wv_prep = np.ascontiguousarray(
            w_qkv[:, vcols].astype(NPBF).reshape(32, 128, 512).transpose(1, 0, 2)
        ).reshape(128, 32 * 512)

        wout_prep = np.ascontiguousarray(
            w_out[c * DPC:(c + 1) * DPC, :].astype(NPBF)
            .reshape(4, 128, H).transpose(1, 0, 2))
        in_maps.append(dict(shared, wqkv=wqkv_prep, wv=wv_prep,
                            wout=wout_prep))
    return in_maps


def kernel(hidden_states, position_ids, attention_mask, w_qkv, w_out):
    global LAST_EXEC_NS
    nc = _get_nc()
    in_maps = _host_prep(hidden_states, position_ids, attention_mask,
                         w_qkv, w_out)
    res = run_bass_kernel_spmd(nc, in_maps, core_ids=list(range(N_CORES)))
    LAST_EXEC_NS = res.exec_time_ns
    out = res.results[0]["out"].astype(np.float32)
    for c in range(1, N_CORES):
        out = out + res.results[c]["out"].astype(np.float32)
    return out.reshape(B, S, H)



# revision 39
# speedup vs baseline: 1.0188x; 1.0188x over previous
"""CodeGen-style attention block, tensor-parallel over heads on 8 Trainium2 cores.

v2: all-bf16 storage/matmuls (f32 PSUM accumulate), serial phase structure
P0 -> A0 -> P1 -> A1 -> OP:
  - P(b): single-pass QKV projection for batch b, 256-token windows, all 12
    weight column-tiles resident (bf16 halves SBUF; FWL hides LDWEIGHTS).
    k and v are written straight into persistent SBUF tiles (no DRAM
    round-trip); q goes to DRAM and is read back per query-block.  v is
    computed directly in [tok, d] form by swapping the matmul operands
    (hsT chunk stationary, v-weights moving) - no PE transposes.
  - A(b): causal attention for this core's 2 heads.  Scores/exp/AV/den with
    diagonal narrowing (only the live column range of diagonal k-chunks is
    computed) and depth-2 software pipelining of score vs AV matmuls in the
    PE stream.  Normalization is deferred: unnormalized AV sums (bf16) and
    softmax denominators (f32) are written to DRAM.
  - OP: out-projection.  Loads AV/den back, reciprocal (fast approx) +
    normalize on DVE off the critical path, then the w_out row-slice matmuls.
Host sums the 8 partial [tokens, H] outputs (out-proj head contraction).
"""

import sys
import types
from contextlib import ExitStack

import numpy as np
import ml_dtypes

import concourse.bacc as bacc
import concourse.mybir as mybir
import concourse.tile as tile
from concourse.bass_utils import run_bass_kernel_spmd

try:
    import antenv.axon_hooks  # noqa: F401
except ImportError:
    _stub = types.ModuleType("antenv.axon_hooks")
    _stub.get_axon_ntff_profile_hook = lambda: None
    _stub.set_axon_ntff_profile_hook = lambda h: None
    sys.modules.setdefault("antenv.axon_hooks", _stub)

F32 = mybir.dt.float32
BF = mybir.dt.bfloat16
AF = mybir.ActivationFunctionType
NPBF = ml_dtypes.bfloat16

B, S, H = 2, 2048, 4096
N_HEAD, HEAD_DIM, ROT = 16, 256, 64
MAX_POS = 2048
TOK = B * S            # 4096
N_CORES = 8
HPC = N_HEAD // N_CORES  # heads per core = 2
DPC = HPC * HEAD_DIM     # dims per core = 512
NEG = -30000.0
NW = 16                  # 256-token windows

LAST_EXEC_NS = None
_NC_CACHE = []


def _build():
    nc = bacc.Bacc("TRN2", target_bir_lowering=False, debug=False,
                   num_devices=N_CORES)

    # [w, p(H-chunk), hc*256 + t]: hidden^T window tiles
    hst_d = nc.dram_tensor("hst", [NW, 128, 32 * 256], BF, kind="ExternalInput")
    # [oc, p(H-chunk), hc*128 + d]: q ocs 0..3 then k ocs 4..7 (stationary)
    wqkv_d = nc.dram_tensor("wqkv", [8, 128, 32 * 128], BF, kind="ExternalInput")
    # [p(H-chunk), hc*512 + vd]: v weights as the moving operand
    wv_d = nc.dram_tensor("wv", [128, 32 * 512], BF, kind="ExternalInput")
    rope_d = nc.dram_tensor("rope", [128, TOK], F32, kind="ExternalInput")
    rt_d = nc.dram_tensor("rt", [64, 64], BF, kind="ExternalInput")
    onm_d = nc.dram_tensor("onesm", [128, 128], BF, kind="ExternalInput")
    tri_d = nc.dram_tensor("tri", [128, 128], F32, kind="ExternalInput")
    kb_d = nc.dram_tensor("kb", [128, 32], F32, kind="ExternalInput")
    # [p, ci(=2*hl+dc), h]: per-core w_out row-slice
    wout_d = nc.dram_tensor("wout", [128, 4, H], BF, kind="ExternalInput")
    out_d = nc.dram_tensor("out", [TOK, H], BF, kind="ExternalOutput")

    with tile.TileContext(nc) as tc:
        with ExitStack() as st0:
            ec0 = st0.enter_context
            dram_pool = ec0(tc.tile_pool(name="dram", bufs=1, space="DRAM"))
            # q_d[(b, qt)]: [128, oc, 512] -- per-qt tiles so an attention
            # block's load only RAW-depends on its own two windows' stores
            q_d = {}
            for b in range(B):
                for qt in range(4):
                    q_d[(b, qt)] = dram_pool.tile(
                        [128, 4, 512], BF, tag=f"qd{b}{qt}", name=f"qd{b}{qt}")
            # b=0 only: b=1 keeps avc/den resident in SBUF through OP
            avc_d = {}
            den_d = {}
            ao_d = {}
            for qt in range(4):
                den_d[(0, qt)] = dram_pool.tile(
                    [128, 2, 512], F32, tag=f"dnd0{qt}", name=f"dnd0{qt}")
                avc_d[(0, qt)] = dram_pool.tile(
                    [128, 4, 512], BF, tag=f"avd0{qt}", name=f"avd0{qt}")
                # b=0 attn output, normalized during P1 (DVE/gpsimd idle
                # there) so A1/OP never spend reciprocal work on b=0
                ao_d[(0, qt)] = dram_pool.tile(
                    [128, 4, 512], BF, tag=f"aod0{qt}", name=f"aod0{qt}")

            # constants (small) -- on the scalar HWDGE queue so the sync
            # queue starts on the weight tiles immediately
            c0 = ec0(tc.tile_pool(name="consts", bufs=1))
            rt_sb = c0.tile([64, 64], BF)
            nc.scalar.dma_start(rt_sb[:], rt_d[:])
            kb_sb = c0.tile([128, 32], F32)
            nc.scalar.dma_start(kb_sb[:], kb_d[:])
            tri_sb = c0.tile([128, 128], F32)
            nc.scalar.dma_start(tri_sb[:], tri_d[:])
            onm_sb = c0.tile([128, 128], BF)
            nc.scalar.dma_start(onm_sb[:], onm_d[:])
            dummy_sb = c0.tile([1, 8], BF)

            # persistent per-batch attention inputs (reused b0 -> b1)
            kvp = ec0(tc.tile_pool(name="kv", bufs=1))
            kts = {(hl, dc): kvp.tile([128, 2048], BF, tag=f"kt{hl}{dc}",
                                      name=f"kt{hl}{dc}")
                   for hl in range(2) for dc in range(2)}
            vh = kvp.tile([128, 16 * 512], BF, tag="vh", name="vh")

            # ---------------- attention machinery ----------------
            qpool = ec0(tc.tile_pool(name="qq", bufs=2))

            def q_dmas(b, qt, hl):
                q = qpool.tile([128, 2, 512], BF, tag="q")
                nc.sync.dma_start(q[:], q_d[(b, qt)][:, 2 * hl:2 * hl + 2, :])
                return q

            def attn_phase(ast, b, preq=None, hook=None, sink=None):
                ec = ast.enter_context
                # 8 PSUM banks: sc 4 + av 2x2.  av double-buffered so a new
                # block's AV matmuls never wait on the previous block's DVE
                # drain; den tiles borrow a slot from the sc/av rings (their
                # lifetime is one matmul + one copy).
                scpool = ec(tc.tile_pool(name=f"sc{b}", bufs=4, space="PSUM"))
                avpool = ec(tc.tile_pool(name=f"av{b}", bufs=2, space="PSUM"))
                expool = ec(tc.tile_pool(name=f"ex{b}", bufs=6))
                aspool = ec(tc.tile_pool(name=f"as{b}", bufs=1))
                xspool = ec(tc.tile_pool(name=f"xs{b}", bufs=2))

                blocks = [(qt, hl) for qt in range(4) for hl in range(2)]
                qtiles = {}

                def load_q(blk):
                    qtiles[blk] = q_dmas(b, blk[0], blk[1])

                if preq is not None:
                    qtiles[blocks[0]] = preq
                else:
                    load_q(blocks[0])

                # cross-block pipelined stream: sc/tri/exp run LAG chunks
                # ahead of the av/den matmuls, flowing across block
                # boundaries so the PE never drains at a block switch.
                LAG = 4
                pipe = []  # (emit_avden, fin_or_None)

                def pump():
                    emit, fin = pipe.pop(0)
                    emit()
                    if fin is not None:
                        fin()

                for bi, blk in enumerate(blocks):
                    qt, hl = blk
                    if hook is not None:
                        hook(bi)
                    if bi + 1 < len(blocks):
                        load_q(blocks[bi + 1])
                    qs = qtiles.pop(blk)
                    nkc = 4 * qt + 4
                    # the phase's last block computes den with per-chunk PE
                    # matmuls (the old hybrid): the phase-end drain must not
                    # serialize on the DVE exsum chain
                    legacy = (bi == len(blocks) - 1)
                    av0 = avpool.tile([128, 512], F32, tag="av0")
                    av1 = avpool.tile([128, 512], F32, tag="av1")
                    # exp chunks are elementwise-accumulated on the DVE; a
                    # single deferred den matmul per block folds the
                    # partition sum.
                    exsum = xspool.tile([128, 512], F32, tag="xs")
                    exsum_bf = xspool.tile([128, 512], BF, tag="xsb")
                    den_box = [None]

                    def store_den(den, qt=qt, hl=hl):
                        if sink is not None:
                            _, densb = sink
                            nc.vector.tensor_copy(
                                densb[qt][:, hl, :], den[:])
                        else:
                            dns = aspool.tile([128, 512], F32, tag="dns")
                            nc.vector.tensor_copy(dns[:], den[:])
                            nc.gpsimd.dma_start(
                                den_d[(b, qt)][:, hl, :], dns[:])

                    def make_fin_av(qt=qt, hl=hl, av0=av0, av1=av1,
                                    legacy=legacy, den_box=den_box):
                        def fin():
                            if sink is not None:
                                avsb, _ = sink
                                for dc, av in ((0, av0), (1, av1)):
                                    nc.vector.tensor_copy(
                                        avsb[qt][:, 2 * hl + dc, :], av[:])
                            else:
                                avs = aspool.tile([128, 2, 512], BF,
                                                  tag="avs")
                                for dc, av in ((0, av0), (1, av1)):
                                    nc.vector.tensor_copy(
                                        avs[:, dc, :], av[:])
                                nc.gpsimd.dma_start(
                                    avc_d[(b, qt)][:, 2 * hl:2 * hl + 2, :],
                                    avs[:])
                            if legacy:
                                store_den(den_box[0], qt=qt, hl=hl)
                        return fin

                    def make_den(qt=qt, hl=hl, exsum_bf=exsum_bf):
                        # deferred a pipe slot past the block's last chunk so
                        # the DVE exsum chain never gates the PE
                        def emit():
                            den = scpool.tile([128, 512], F32, tag="sc")
                            nc.tensor.matmul(den[:], onm_sb[:], exsum_bf[:],
                                             start=True, stop=True)
                            store_den(den, qt=qt, hl=hl)
                        return emit

                    for kc in range(nkc):
                        j = kc - 4 * qt
                        n0 = 128 * j if j > 0 else 0
                        sc = scpool.tile([128, 512], F32, tag="sc")
                        nc.tensor.matmul(
                            sc[:, n0:512], kts[(hl, 0)][:, kc * 128:(kc + 1) * 128],
                            qs[:, 0, n0:512], start=True, stop=False)
                        nc.tensor.matmul(
                            sc[:, n0:512], kts[(hl, 1)][:, kc * 128:(kc + 1) * 128],
                            qs[:, 1, n0:512], start=False, stop=True)
                        if j >= 0:
                            # causal triangle within the diagonal 128-col band
                            nc.vector.tensor_add(
                                sc[:, n0:n0 + 128], sc[:, n0:n0 + 128], tri_sb[:])
                        ex = expool.tile([128, 512], BF, tag="ex")
                        nc.scalar.activation(
                            ex[:, n0:512], sc[:, n0:512], AF.Exp,
                            scale=1.0 / 16.0,
                            bias=kb_sb[:, b * 16 + kc:b * 16 + kc + 1])
                        if not legacy or j < 0:
                            if kc == 0:
                                nc.vector.tensor_copy(exsum[:], ex[:])
                            else:
                                nc.vector.tensor_add(
                                    exsum[:, n0:512], exsum[:, n0:512],
                                    ex[:, n0:512])
                        if (kc == nkc - 1 and not legacy) or (
                                legacy and kc == 4 * qt - 1):
                            nc.vector.tensor_copy(exsum_bf[:], exsum[:])

                        def emit(ex=ex, n0=n0, kc=kc, nkc=nkc, hl=hl, qt=qt,
                                 av0=av0, av1=av1, legacy=legacy,
                                 den_box=den_box, exsum_bf=exsum_bf):
                            st, sp = (kc == 0), (kc == nkc - 1)
                            base = kc * 512 + hl * 256
                            nc.tensor.matmul(
                                av0[:, n0:512], vh[:, base:base + 128],
                                ex[:, n0:512], start=st, stop=sp)
                            nc.tensor.matmul(
                                av1[:, n0:512], vh[:, base + 128:base + 256],
                                ex[:, n0:512], start=st, stop=sp)
                            jj = kc - 4 * qt
                            if legacy and jj >= 0:
                                if jj == 0:
                                    den_box[0] = avpool.tile(
                                        [128, 512], F32, tag="av0",
                                        name="denleg")
                                    nc.tensor.matmul(
                                        den_box[0][:], onm_sb[:],
                                        exsum_bf[:], start=True, stop=False)
                                nc.tensor.matmul(
                                    den_box[0][:, n0:512], onm_sb[:],
                                    ex[:, n0:512], start=False, stop=sp)

                        pipe.append(
                            (emit, make_fin_av() if kc == nkc - 1 else None))
                        if len(pipe) > LAG:
                            pump()
                    if not legacy:
                        pipe.append((make_den(), None))
                        if len(pipe) > LAG:
                            pump()
                while pipe:
                    pump()

            # ---------------- phases P0, A0, P1 ----------------
            with ExitStack() as stw:
                ecw = stw.enter_context
                wpool = ecw(tc.tile_pool(name="w", bufs=1))
                htpool = ecw(tc.tile_pool(name="ht", bufs=2))
                ropool = ecw(tc.tile_pool(name="ro", bufs=2))
                spool = ecw(tc.tile_pool(name="stage", bufs=4))
                tpool = ecw(tc.tile_pool(name="rott", bufs=2))

                def ht_load(w, strips=1):
                    t = htpool.tile([128, 32 * 256], BF, tag="ht", name="ht")
                    step = 32 // strips
                    for s in range(strips):
                        cs = slice(s * step * 256, (s + 1) * step * 256)
                        nc.sync.dma_start(t[:, cs], hst_d[w][:, cs])
                    return t

                def rope_load(w):
                    t = ropool.tile([128, 256], F32, tag="ro", name="ro")
                    nc.sync.dma_start(t[:], rope_d[:, w * 256:(w + 1) * 256])
                    return t

                # startup cascade: the first k-chain consumes wt4/ht in
                # hc order, so stripe both and interleave the DMAs to get
                # the first matmul issuing as early as possible.
                wts = {}
                wt4 = wpool.tile([128, 32 * 128], BF, tag="w4", name="wt4")
                wts[4] = wt4
                ht0 = htpool.tile([128, 32 * 256], BF, tag="ht", name="ht")
                # small leading strips so the first k-chain's leading hc
                # tiles land (and the chain starts) as early as possible
                nc.sync.dma_start(wt4[:, 0:1024], wqkv_d[4][:, 0:1024])
                nc.sync.dma_start(ht0[:, 0:2048], hst_d[0][:, 0:2048])
                nc.sync.dma_start(wt4[:, 1024:4096], wqkv_d[4][:, 1024:4096])
                nc.sync.dma_start(ht0[:, 2048:8192], hst_d[0][:, 2048:8192])
                for kq in range(1, 4):
                    wt = wpool.tile([128, 32 * 128], BF, tag=f"w{4 + kq}",
                                    name=f"wt{4 + kq}")
                    nc.sync.dma_start(wt[:], wqkv_d[4 + kq])
                    wts[4 + kq] = wt
                pre = (ht0, rope_load(0))
                # q weights BEFORE wv: window 0 runs k -> q -> v so the wv
                # transfer hides behind the q chains
                for oc in range(4):
                    wt = wpool.tile([128, 32 * 128], BF, tag=f"w{oc}",
                                    name=f"wt{oc}")
                    nc.sync.dma_start(wt[:], wqkv_d[oc])
                    wts[oc] = wt
                wv_sb = wpool.tile([128, 32 * 512], BF, tag="wv", name="wv")
                nc.sync.dma_start(wv_sb[:], wv_d[:])
                # preload the exp ACT table (emitted after the const-dma
                # dispatches so it doesn't delay the rt/kb transfers)
                nc.scalar.activation(dummy_sb[:], kb_sb[0:1, 0:8], AF.Exp)

                # warm the PE (HAM) under the startup DMA backlog
                with ExitStack() as wst:
                    wps = wst.enter_context(
                        tc.tile_pool(name="warm", bufs=1, space="PSUM"))
                    warm = wps.tile([64, 64], F32)
                    for i in range(64):
                        nc.tensor.matmul(warm[:], rt_sb[:], rt_sb[:],
                                         start=(i == 0), stop=(i == 63))

                # b=0 out-proj normalization, precomputed under P1:
                # ao = avc * (1/den), all off the PE / critical engines
                def norm_prep0(qt, nppool):
                    # loads ride the sync queue: their data lands a window
                    # later, so the scheduler cannot hoist the DVE recips
                    # into the A0 drain window
                    al = nppool.tile([128, 4, 512], BF, tag="nal")
                    nc.sync.dma_start(al[:], avc_d[(0, qt)][:])
                    dn = nppool.tile([128, 2, 512], F32, tag="ndn")
                    nc.sync.dma_start(dn[:], den_d[(0, qt)][:])
                    ao = nppool.tile([128, 4, 512], BF, tag="nao")
                    # 1/den = exp(-ln(den)) on the ACT engine (idle all
                    # through P1; DVE must stay clear of this work or the
                    # scheduler drags it into the A0 drain / P1 pipeline).
                    # Both Lns then both Exps to minimize ACT table switches.
                    lg = nppool.tile([128, 2, 512], F32, tag="nlg")
                    rc = nppool.tile([128, 2, 512], F32, tag="nrc")
                    for hl in range(2):
                        nc.scalar.activation(lg[:, hl, :], dn[:, hl, :],
                                             AF.Ln)
                    for hl in range(2):
                        nc.scalar.activation(rc[:, hl, :], lg[:, hl, :],
                                             AF.Exp, scale=-1.0)
                    for hl in range(2):
                        for dc in range(2):
                            nc.gpsimd.tensor_mul(
                                ao[:, 2 * hl + dc, :],
                                al[:, 2 * hl + dc, :], rc[:, hl, :])
                    nc.gpsimd.dma_start(ao_d[(0, qt)][:], ao[:])

                def proj_phase(pst, b, pre):
                    """QKV projection for batch b (windows 8b .. 8b+7)."""
                    ec = pst.enter_context
                    apool = ec(tc.tile_pool(name=f"pa{b}", bufs=3, space="PSUM"))
                    rpool = ec(tc.tile_pool(name=f"pr{b}", bufs=2, space="PSUM"))
                    nppool = (ec(tc.tile_pool(name="nprep", bufs=1))
                              if b == 1 else None)
                    ht, ro = pre
                    nxt = None
                    preq = None
                    for wl in range(8):
                        w = 8 * b + wl
                        if wl > 0:
                            ht, ro = next_pre
                        if wl == 2:
                            # early q loads for this batch's first attn block
                            preq = q_dmas(b, 0, 0)

                        def rotary(dst, acc, stage_rows, ro=ro):
                            # dst[0:64] <- acc*cos + rotate_every_two(acc)*sin
                            rp = rpool.tile([64, 256], F32)
                            nc.tensor.matmul(rp[:], rt_sb[:], stage_rows,
                                             start=True, stop=True)
                            t1 = tpool.tile([64, 256], F32, tag="t1")
                            nc.vector.tensor_mul(t1[:], acc[0:64, :], ro[0:64, :])
                            t2 = tpool.tile([64, 256], F32, tag="t2")
                            nc.vector.tensor_mul(t2[:], rp[:], ro[64:128, :])
                            nc.vector.tensor_add(dst, t1[:], t2[:])

                        # k ocs (stationary weights, transposed domain)
                        for kq in range(4):
                            hl, dc = kq // 2, kq % 2
                            acc = apool.tile([128, 256], F32, tag="acc")
                            for hc in range(32):
                                nc.tensor.matmul(
                                    acc[:],
                                    wts[4 + kq][:, hc * 128:(hc + 1) * 128],
                                    ht[:, hc * 256:(hc + 1) * 256],
                                    start=(hc == 0), stop=(hc == 31))
                            dst = kts[(hl, dc)][:, wl * 256:(wl + 1) * 256]
                            nc.vector.tensor_copy(dst, acc[:])
                            if dc == 0:
                                rotary(dst[0:64, :], acc, dst[0:64, :])
                            if kq == 1:
                                # prefetch next window under this one
                                if wl < 7:
                                    next_pre = (ht_load(w + 1), rope_load(w + 1))
                                elif b == 0:
                                    nxt = (ht_load(8), rope_load(8))
                        def v_chains():
                            # v (hsT chunks stationary, v-weights moving
                            # -> [tok, d])
                            for ts in range(2):
                                acc = apool.tile([128, 512], F32, tag="acc")
                                for hc in range(32):
                                    nc.tensor.matmul(
                                        acc[:],
                                        ht[:, hc * 256 + ts * 128:
                                           hc * 256 + (ts + 1) * 128],
                                        wv_sb[:, hc * 512:(hc + 1) * 512],
                                        start=(hc == 0), stop=(hc == 31))
                                kc = wl * 2 + ts
                                nc.vector.tensor_copy(
                                    vh[:, kc * 512:(kc + 1) * 512], acc[:])

                        def q_chains():
                            # q ocs -> one staged tile -> single DMA
                            stage = spool.tile([128, 4, 256], BF, tag="qs")
                            for oc in range(4):
                                hl, dc = oc // 2, oc % 2
                                acc = apool.tile([128, 256], F32, tag="acc")
                                for hc in range(32):
                                    nc.tensor.matmul(
                                        acc[:],
                                        wts[oc][:, hc * 128:(hc + 1) * 128],
                                        ht[:, hc * 256:(hc + 1) * 256],
                                        start=(hc == 0), stop=(hc == 31))
                                dst = stage[:, oc, :]
                                nc.vector.tensor_copy(dst, acc[:])
                                if dc == 0:
                                    rotary(dst[0:64, :], acc, dst[0:64, :])
                            nc.sync.dma_start(
                                q_d[(b, wl // 2)][:, :, (wl % 2) * 256:
                                                  (wl % 2) * 256 + 256],
                                stage[:])

                        if b == 0 and wl == 0:
                            q_chains()
                            v_chains()
                        else:
                            v_chains()
                            q_chains()
                        if b == 1 and 2 <= wl <= 5:
                            norm_prep0(wl - 2, nppool)
                    return nxt, preq

                with ExitStack() as pst:
                    pre1, preq0 = proj_phase(pst, 0, pre)
                with ExitStack() as ast:
                    attn_phase(ast, 0, preq=preq0)
                with ExitStack() as pst:
                    _, preq1 = proj_phase(pst, 1, pre1)
            # w_scope closed: projection SBUF freed

            with ExitStack() as st3:
                ec3 = st3.enter_context
                c3 = ec3(tc.tile_pool(name="wo", bufs=1))
                wout_sb = c3.tile([128, 4, H], BF)
                # b=1 avc/den never leave SBUF
                avsb = {qt: c3.tile([128, 4, 512], BF, tag=f"avsb{qt}",
                                    name=f"avsb{qt}") for qt in range(4)}
                densb = {qt: c3.tile([128, 2, 512], F32, tag=f"dnsb{qt}",
                                     name=f"dnsb{qt}") for qt in range(4)}

                # ---------------- out-projection ----------------
                alpool = ec3(tc.tile_pool(name="al", bufs=3))
                recpool = ec3(tc.tile_pool(name="rec", bufs=3))
                scrpool = ec3(tc.tile_pool(name="scr", bufs=2))
                aopool = ec3(tc.tile_pool(name="ao", bufs=4))
                ospool = ec3(tc.tile_pool(name="os", bufs=2))

                aos_all = {}

                def prep(b, qt, eng=None):
                    if b == 0:
                        # normalization already done under P1 -- pure load.
                        # sync queue: its completion signal reaches the PE
                        # directly instead of riding the busy ACT stream
                        aol = alpool.tile([128, 4, 512], BF, tag="al")
                        nc.sync.dma_start(aol[:], ao_d[(0, qt)][:])
                        aos_all[(b, qt)] = lambda hl, dc: aol[:, 2 * hl + dc, :]
                        return
                    al, dn = avsb[qt], densb[qt]
                    aos = {}
                    for hl in range(2):
                        rc = recpool.tile([128, 512], F32, tag=f"rc{hl}")
                        scr = scrpool.tile([128, 512], F32, tag="scr")
                        nc.vector.reciprocal_approx_accurate(
                            out=rc[:], in_=dn[:, hl, :], scratch=scr[:])
                        for dc in range(2):
                            ao = aopool.tile([128, 512], BF, tag=f"ao{hl}{dc}")
                            (eng or nc.vector).tensor_mul(
                                ao[:], al[:, 2 * hl + dc, :], rc[:])
                            aos[(hl, dc)] = ao
                    aos_all[(b, qt)] = lambda hl, dc: aos[(hl, dc)][:]

                def op_block(b, qt):
                    aos = aos_all.pop((b, qt))
                    for tc_ in range(4):
                        stage = ospool.tile([128, 8 * 512], BF, tag="os")
                        for ht_ in range(8):
                            op = oppool.tile([128, 512], F32, tag="op")
                            for ci, (hl, dc) in enumerate(
                                    ((0, 0), (0, 1), (1, 0), (1, 1))):
                                nc.tensor.matmul(
                                    op[:],
                                    aos(hl, dc)[:, tc_ * 128:(tc_ + 1) * 128],
                                    wout_sb[:, 2 * hl + dc,
                                            ht_ * 512:(ht_ + 1) * 512],
                                    start=(ci == 0), stop=(ci == 3))
                            dst = stage[:, ht_ * 512:(ht_ + 1) * 512]
                            if ht_ % 2 == 0:
                                nc.vector.tensor_copy(dst, op[:])
                            else:
                                nc.scalar.copy(dst, op[:])
                        r0 = b * 2048 + qt * 512 + tc_ * 128
                        nc.sync.dma_start(out_d[r0:r0 + 128, :], stage[:])

                def a1_hook(bi):
                    # all hook work is pure DMA prefetch on the sync queue
                    # (idle in A1 but for one q load per block), starting at
                    # bi=1 so the first dispatch's SBUF anti-dependency (on
                    # P1's last q-store) is already clear
                    if bi == 1:
                        prep(0, 0)
                    elif bi in (2, 3, 4, 5):
                        i = bi - 2
                        nc.sync.dma_start(wout_sb[:, i, :],
                                          wout_d[:, i, :])
                    elif bi in (6, 7):
                        prep(0, bi - 5)

                with ExitStack() as ast:
                    attn_phase(ast, 1, preq=preq1, hook=a1_hook,
                               sink=(avsb, densb))

                oppool = ec3(tc.tile_pool(name="op", bufs=4, space="PSUM"))
                obs = [(b, qt) for b in range(B) for qt in range(4)]
                for i, ob in enumerate(obs):
                    op_block(*ob)
                    if i + 3 < len(obs):
                        prep(*obs[i + 3])
    nc.compile()
    return nc


def _get_nc():
    if not _NC_CACHE:
        _NC_CACHE.append(_build())
    return _NC_CACHE[0]


def _host_prep(hidden_states, position_ids, attention_mask, w_qkv, w_out):
    hid = np.ascontiguousarray(np.asarray(hidden_states, np.float32)).reshape(TOK, H)
    w_qkv = np.asarray(w_qkv, np.float32)
    w_out = np.asarray(w_out, np.float32)
    pos = np.asarray(position_ids).astype(np.int64)
    am = np.asarray(attention_mask).reshape(B, S).astype(bool)

    # hsT window tiles [w, p, hc*256 + t]
    hst = np.ascontiguousarray(
        hid.astype(NPBF).reshape(NW, 256, 32, 128).transpose(0, 3, 2, 1)
    ).reshape(NW, 128, 32 * 256)

    # rotary tables, matching reference.create_sinusoidal_positions
    inv_freq = 1.0 / 10000 ** (np.arange(0, ROT, 2) / ROT)
    si = np.einsum('i,j->ij', np.arange(MAX_POS), inv_freq).astype('float32')
    emb = np.concatenate([np.sin(si), np.cos(si)], axis=-1)  # [2048, 64]
    sincos = emb[pos]                    # [B, S, 64]
    sin_rep = np.repeat(sincos[..., :ROT // 2], 2, axis=2)   # [B, S, 64]
    cos_rep = np.repeat(sincos[..., ROT // 2:], 2, axis=2)
    rope = np.empty((128, TOK), np.float32)
    rope[0:64] = cos_rep.reshape(TOK, 64).T
    rope[64:128] = sin_rep.reshape(TOK, 64).T

    rt = np.zeros((64, 64), np.float32)
    rt[np.arange(1, 64, 2), np.arange(0, 64, 2)] = -1.0
    rt[np.arange(0, 64, 2), np.arange(1, 64, 2)] = 1.0

    onesm = np.ones((128, 128), np.float32)

    p_idx = np.arange(128)[:, None]
    c_idx = np.arange(128)[None, :]
    tri = np.where(p_idx <= c_idx, 0.0, NEG).astype(np.float32)

    kb = np.where(am.reshape(B, 16, 128), 0.0, NEG).astype(
        np.float32).transpose(2, 0, 1).reshape(128, 32)
    kb = np.ascontiguousarray(kb)

    shared = dict(hst=hst, rope=rope, rt=rt.astype(NPBF),
                  onesm=onesm.astype(NPBF), tri=tri, kb=kb)

    in_maps = []
    for c in range(N_CORES):
        # q ocs 0..3 then k ocs 4..7; fused layout per mp-group is (q, v, k)
        occols = []
        for part in (0, 2):  # 0 = query, 2 = key
            for hl in range(HPC):
                h = HPC * c + hl
                base = (h // 4) * 3072 + part * 1024 + (h % 4) * 256
                occols.append(np.arange(base, base + 256))
        occols = np.concatenate(occols)  # [1024] = q(512) | k(512)
        wslice = w_qkv[:, occols].astype(NPBF)  # [4096, 1024]
        wqkv_prep = np.ascontiguousarray(
            wslice.reshape(32, 128, 8, 128).transpose(2, 1, 0, 3)
        ).reshape(8, 128, 32 * 128)

        vcols = []
        for hl in range(HPC):
            h = HPC * c + hl
            base = (h // 4) * 3072 + 1 * 1024 + (h % 4) * 256
            vcols.append(np.arange(base, base + 256))
        vcols = np.concatenate(vcols)    # [512]
        wv_prep = np.ascontiguousarray(
            w_qkv[:, vcols].astype(NPBF).reshape(32, 128, 512).transpose(1, 0, 2)
        ).reshape(128, 32 * 512)

        wout_prep = np.ascontiguousarray(
            w_out[c * DPC:(c + 1) * DPC, :].astype(NPBF)
            .reshape(4, 128, H).transpose(1, 0, 2))
        in_maps.append(dict(shared, wqkv=wqkv_prep, wv=wv_prep,
                            wout=wout_prep))
    return in_maps


def kernel(hidden_states, position_ids, attention_mask, w_qkv, w_out):
    global LAST_EXEC_NS
    nc = _get_nc()
    in_maps = _host_prep(hidden_states, position_ids, attention_mask,
                         w_qkv, w_out)
    res = run_bass_kernel_spmd(nc, in_maps, core_ids=list(range(N_CORES)))
    LAST_EXEC_NS = res.exec_time_ns
    out = res.results[0]["out"].astype(np.float32)
    for c in range(1, N_CORES):
        out = out + res.results[c]["out"].astype(np.float32)
    return out.reshape(B, S, H)

